# revision 15
# baseline (speedup 1.0000x reference)
"""Trainium2 Bass kernel for CompressedGlobalAttention (bf16 rewrite).

Problem (hardcoded shapes from the reference):
  x: (2, 8192, 1024) fp32, local_window_start=4096, 16 heads x 64 dim,
  compression ratio 8 -> 512 avg-pooled KV "pools" from the first 4096
  tokens of each batch.  out = softmax(mask(q @ k_c^T)) @ v_c projected.

Sharding (8 cores): core = b*4 + qi handles batch b, query rows
[qi*2048, (qi+1)*2048).  Each core recomputes the (cheap) pooled k/v for
its batch locally; outputs are disjoint row blocks -> no cross-core
reduction.  All mask structure is passed as per-core *data* so every
core runs the identical SPMD program.

Layout strategy (all matmuls bf16, 1 PE cycle/row):
  - host supplies x pre-transposed per core: xqT [1024, 2048] (query
    block, channels-major) and xpT [1024, 4096] (pooling source,
    channels-major, pool-permuted for qi==1) so no PE transposes are
    needed anywhere.
  - pooling = free-dim avg-pool of xpT on the vector engine -> pooledT
    [1024, 512] directly in the layout the k/v projections consume.
  - score layout is [pool, seq]; softmax sums come from a ones-column
    in the augmented v; attn@v needs no transposes.
  - fully-masked (pool-chunk, seq-tile) pairs are killed by a per-
    partition bias column fed to the exp() activation (-1e9 -> exp=0).
  - the causal boundary only crosses a 64-pool strip per 512-row seq
    tile; a per-core pool permutation (folded into the host-side xpT
    layout) pins that strip to a static partition range, where a
    multiplicative {0,1} bf16 mask is applied to e after exp.
  - rows 0..7 of each batch attend to nothing (reference: uniform
    softmax over all pools); the kernel produces zeros there and the
    host overwrites them with the analytic uniform-attention value.
"""

import os
import sys

import numpy as np

NUM_HEADS = 16
HEAD_DIM = 64
RATIO = 8
B, S, D = 2, 8192, 1024
LWS = 4096
NPOOL = LWS // RATIO        # 512
SQ = S // 4                 # 2048 query rows per core
N_CORES = 8
ST = 512                    # seq tile (free dim) in phase B
NST = SQ // ST              # 4 seq tiles per core
NEG = -1.0e9

_RUNNER = None


def _ensure_path():
    for p in ("/opt/trn_rl_repo",):
        if p not in sys.path and os.path.isdir(p):
            sys.path.insert(0, p)


def build_program():
    """Build the Bass/Tile SPMD program (same for all 8 cores)."""
    _ensure_path()
    import concourse.bass as bass
    import concourse.mybir as mybir
    import concourse.tile as tile
    from contextlib import ExitStack

    f32 = mybir.dt.float32
    bf16 = mybir.dt.bfloat16
    Exp = mybir.ActivationFunctionType.Exp

    nc = bass.Bass("TRN2", target_bir_lowering=False, debug=False)

    xqT = nc.declare_dram_parameter("xqT", [D, SQ], bf16, isOutput=False)
    xpT = nc.declare_dram_parameter("xpT", [D, LWS], bf16, isOutput=False)
    wq = nc.declare_dram_parameter("wq", [D, D], bf16, isOutput=False)
    wk = nc.declare_dram_parameter("wk", [D, D], bf16, isOutput=False)
    wv = nc.declare_dram_parameter("wv", [D, D], bf16, isOutput=False)
    wo = nc.declare_dram_parameter("wo", [D, D], bf16, isOutput=False)
    bq2 = nc.declare_dram_parameter("bq2", [128, 8], f32, isOutput=False)
    bk2 = nc.declare_dram_parameter("bk2", [128, 8], f32, isOutput=False)
    bvr = nc.declare_dram_parameter("bvr", [1, D], bf16, isOutput=False)
    bor = nc.declare_dram_parameter("bor", [1, D], bf16, isOutput=False)
    hsd = nc.declare_dram_parameter("headsel", [16, D], bf16, isOutput=False)
    dgd = nc.declare_dram_parameter("diagmask", [128, NST * ST], bf16, isOutput=False)
    bmd = nc.declare_dram_parameter("biasmask", [128, 16], f32, isOutput=False)
    yout = nc.declare_dram_parameter("y", [SQ, D], f32, isOutput=True)

    with tile.TileContext(nc) as tc, ExitStack() as top:
        # ---------------- persistent pools ----------------
        consts = top.enter_context(tc.tile_pool(name="consts", bufs=1))
        kTp = top.enter_context(tc.tile_pool(name="kTp", bufs=1))
        vap = top.enter_context(tc.tile_pool(name="vap", bufs=1))

        headsel = consts.tile([16, D], bf16, name="headsel")
        nc.sync.dma_start(headsel[:], hsd[:, :])
        bq2_sb = consts.tile([128, 8], f32, name="bq2_sb")
        nc.sync.dma_start(bq2_sb[:], bq2[:, :])
        bor_sb = consts.tile([1, D], bf16, name="bor_sb")
        nc.sync.dma_start(bor_sb[:], bor[:, :])
        diag_sb = consts.tile([128, NST * ST], bf16, name="diag_sb")
        nc.sync.dma_start(diag_sb[:], dgd[:, :])
        bias_sb = consts.tile([128, 16], f32, name="bias_sb")
        nc.sync.dma_start(bias_sb[:], bmd[:, :])
        ones1 = consts.tile([1, 128], bf16, name="ones1")
        nc.vector.memset(ones1[:], 1.0)

        kT = [kTp.tile([128, NPOOL], bf16, name=f"kT{j}", tag=f"kT{j}") for j in range(8)]
        vaug = [
            vap.tile([128, NUM_HEADS * (HEAD_DIM + 1)], bf16, name=f"vaug{i}", tag=f"vaug{i}")
            for i in range(4)
        ]

        # ---------------- phase A: pooled k/v ----------------
        with ExitStack() as pa:
            aconsts = pa.enter_context(tc.tile_pool(name="aconsts", bufs=1))
            wkvp = pa.enter_context(tc.tile_pool(name="wkvp", bufs=1))
            xpp = pa.enter_context(tc.tile_pool(name="xpp", bufs=3))
            pltp = pa.enter_context(tc.tile_pool(name="pltp", bufs=1))

            bk2_sb = aconsts.tile([128, 8], f32, name="bk2_sb")
            nc.sync.dma_start(bk2_sb[:], bk2[:, :])
            bvr_sb = aconsts.tile([1, D], bf16, name="bvr_sb")
            nc.sync.dma_start(bvr_sb[:], bvr[:, :])

            wk_sb = [wkvp.tile([128, D], bf16, name=f"wk{m}", tag=f"wk{m}") for m in range(8)]
            wv_sb = [wkvp.tile([128, D], bf16, name=f"wv{m}", tag=f"wv{m}") for m in range(8)]
            for m in range(8):
                nc.sync.dma_start(wk_sb[m][:], wk[m * 128 : (m + 1) * 128, :])
                nc.sync.dma_start(wv_sb[m][:], wv[m * 128 : (m + 1) * 128, :])

            pooledT = [pltp.tile([128, NPOOL], bf16, name=f"pooledT{m}", tag=f"pooledT{m}") for m in range(8)]

            # pooling: free-dim sum-pool (window 8) of xpT channel tiles;
            # the 1/RATIO mean scaling is folded into Wk/Wv on the host
            for m in range(8):
                xt = xpp.tile([128, LWS], bf16, name="xt", tag="xt")
                nc.sync.dma_start(xt[:], xpT[m * 128 : (m + 1) * 128, :])
                with nc.allow_low_precision(reason="bf16 pooling"):
                    nc.vector.tensor_reduce(
                        pooledT[m][:],
                        xt[:].rearrange("p (n w) -> p n w", w=RATIO),
                        axis=mybir.AxisListType.X,
                        op=mybir.AluOpType.add,
                    )

            # kT[j][d, p] = sum_c Wk[c, d] pooledT[c, p] + bk[d]
            with tc.tile_pool(name="kv_ps", bufs=2, space="PSUM") as kv_ps:
                for j in range(8):
                    ps = kv_ps.tile([128, NPOOL], f32, name="ps2", tag="kvps")
                    for m in range(8):
                        nc.tensor.matmul(
                            ps[:],
                            wk_sb[m][:, j * 128 : (j + 1) * 128],
                            pooledT[m][:],
                            start=(m == 0),
                            stop=(m == 7),
                        )
                    nc.scalar.add(kT[j][:], ps[:], bk2_sb[:, j : j + 1])

            # v[p, d] = sum_c pooledT[c, p] Wv[c, d] + bv[d]; augment ones col
            with tc.tile_pool(name="v_ps", bufs=2, space="PSUM") as v_ps:
                for i in range(4):
                    ps = v_ps.tile([128, D], f32, name="ps3", tag="vps")
                    for m in range(8):
                        for h2 in range(2):
                            nc.tensor.matmul(
                                ps[:, h2 * 512 : (h2 + 1) * 512],
                                pooledT[m][:, i * 128 : (i + 1) * 128],
                                wv_sb[m][:, h2 * 512 : (h2 + 1) * 512],
                                start=(m == 0),
                                stop=False,
                            )
                    for h2 in range(2):
                        nc.tensor.matmul(
                            ps[:, h2 * 512 : (h2 + 1) * 512],
                            ones1[:],
                            bvr_sb[:, h2 * 512 : (h2 + 1) * 512],
                            start=False,
                            stop=True,
                        )
                    va = vaug[i][:].rearrange("p (h x) -> p h x", x=HEAD_DIM + 1)
                    nc.vector.tensor_copy(
                        va[:, :, 0:HEAD_DIM],
                        ps[:].rearrange("p (h x) -> p h x", x=HEAD_DIM),
                    )
                    nc.vector.memset(va[:, :, HEAD_DIM : HEAD_DIM + 1], 1.0)

        # ---------------- phase B: attention ----------------
        with ExitStack() as pb:
            wqop = pb.enter_context(tc.tile_pool(name="wqop", bufs=1))
            xTp = pb.enter_context(tc.tile_pool(name="xTp", bufs=2))
            qTp = pb.enter_context(tc.tile_pool(name="qTp", bufs=1))
            ep = pb.enter_context(tc.tile_pool(name="ep", bufs=2))
            oTp = pb.enter_context(tc.tile_pool(name="oTp", bufs=1))
            dnp = pb.enter_context(tc.tile_pool(name="dnp", bufs=1))
            ysp = pb.enter_context(tc.tile_pool(name="ysp", bufs=2))
            psb = pb.enter_context(tc.tile_pool(name="psb", bufs=1, space="PSUM"))

            wq_sb = [wqop.tile([128, D], bf16, name=f"wq{m}", tag=f"wq{m}") for m in range(8)]
            wo_sb = [wqop.tile([128, D], bf16, name=f"wo{j}", tag=f"wo{j}") for j in range(8)]
            for m in range(8):
                nc.sync.dma_start(wq_sb[m][:], wq[m * 128 : (m + 1) * 128, :])
                nc.sync.dma_start(wo_sb[m][:], wo[m * 128 : (m + 1) * 128, :])

            for st in range(NST):
                s0 = st * ST
                # x^T tiles straight from DRAM (host pre-transposed)
                xT = [xTp.tile([128, ST], bf16, name=f"xT{m}", tag=f"xT{m}") for m in range(8)]
                for m in range(8):
                    nc.sync.dma_start(xT[m][:], xqT[m * 128 : (m + 1) * 128, s0 : s0 + ST])
                # q^T[d, s]
                qT = [qTp.tile([128, ST], bf16, name=f"qT{j}", tag=f"qT{j}") for j in range(8)]
                for j in range(8):
                    ps = psb.tile([128, ST], f32, name="qps", tag="qr", bufs=2)
                    for m in range(8):
                        nc.tensor.matmul(
                            ps[:],
                            wq_sb[m][:, j * 128 : (j + 1) * 128],
                            xT[m][:],
                            start=(m == 0),
                            stop=(m == 7),
                        )
                    nc.vector.tensor_scalar_add(qT[j][:], ps[:], bq2_sb[:, j : j + 1])

                oT = [oTp.tile([128, ST], bf16, name=f"oT{j}", tag=f"oT{j}") for j in range(8)]
                dpc = st // 2
                dof = 64 * (st % 2)

                denoms = dnp.tile([16, ST], f32, name="denoms", tag="denoms")
                for h in range(NUM_HEADS):
                    j, r0 = h // 2, 64 * (h % 2)
                    sc = []
                    for pc in range(4):
                        t = psb.tile([128, ST], f32, name=f"sc{pc}", tag=f"pc{pc % 2}", bufs=1)
                        nc.tensor.matmul(
                            t[:],
                            kT[j][r0 : r0 + 64, pc * 128 : (pc + 1) * 128],
                            qT[j][r0 : r0 + 64, :],
                            start=True,
                            stop=True,
                        )
                        sc.append(t)
                    e = []
                    for pc in range(4):
                        et = ep.tile([128, ST], bf16, name=f"e{pc}", tag=f"e{pc}")
                        nc.scalar.activation(
                            et[:],
                            sc[pc][:],
                            Exp,
                            bias=bias_sb[:, st * 4 + pc : st * 4 + pc + 1],
                            scale=1.0 / np.sqrt(HEAD_DIM),
                        )
                        if pc == dpc:
                            # multiplicative {0,1} boundary mask on the strip
                            nc.vector.tensor_mul(
                                et[dof : dof + 64, :],
                                et[dof : dof + 64, :],
                                diag_sb[dof : dof + 64, st * ST : (st + 1) * ST],
                            )
                        e.append(et)
                    oa = psb.tile([HEAD_DIM + 1, ST], f32, name="oa", tag="oa", bufs=2)
                    for pc in range(4):
                        nc.tensor.matmul(
                            oa[:],
                            vaug[pc][:, h * 65 : h * 65 + 65],
                            e[pc][:],
                            start=(pc == 0),
                            stop=(pc == 3),
                        )
                    nc.vector.tensor_copy(oT[j][r0 : r0 + 64, :], oa[0:HEAD_DIM, :])
                    # denom row -> base-0 sbuf tile, clamped, then DMA-gather
                    drow = dnp.tile([1, ST], f32, name="drow", tag="drow", bufs=4)
                    nc.vector.tensor_scalar_max(
                        drow[:], oa[HEAD_DIM : HEAD_DIM + 1, :], 1e-30
                    )
                    nc.sync.dma_start(denoms[h : h + 1, :], drow[:])

                recips = dnp.tile([16, ST], bf16, name="recips", tag="recips")
                with nc.allow_low_precision(reason="bf16 attn normalization"):
                    nc.vector.reciprocal(recips[:], denoms[:])
                for j in range(8):
                    rps = psb.tile([128, ST], f32, name="rps", tag="qr", bufs=2)
                    nc.tensor.matmul(
                        rps[:],
                        headsel[:, j * 128 : (j + 1) * 128],
                        recips[:],
                        start=True,
                        stop=True,
                    )
                    nc.vector.tensor_mul(oT[j][:], oT[j][:], rps[:])

                # final projection y[s, :] = sum O^T.T Wo + bo
                for q4 in range(4):
                    ysb = ysp.tile([128, D], f32, name="ysb", tag="ysb")
                    for hf in range(2):
                        yh = psb.tile([128, 512], f32, name=f"y{hf}", tag=f"pc{hf}", bufs=1)
                        for j in range(8):
                            nc.tensor.matmul(
                                yh[:],
                                oT[j][:, q4 * 128 : (q4 + 1) * 128],
                                wo_sb[j][:, hf * 512 : (hf + 1) * 512],
                                start=(j == 0),
                                stop=False,
                            )
                        nc.tensor.matmul(
                            yh[:],
                            ones1[:],
                            bor_sb[:, hf * 512 : (hf + 1) * 512],
                            start=False,
                            stop=True,
                        )
                        nc.scalar.copy(ysb[:, hf * 512 : (hf + 1) * 512], yh[:])
                    nc.sync.dma_start(
                        yout[s0 + q4 * 128 : s0 + q4 * 128 + 128, :], ysb[:]
                    )

    return nc


# ---------------------------------------------------------------------------
# host side
# ---------------------------------------------------------------------------

def _host_constants():
    headsel = np.zeros((16, D), np.float32)
    for h in range(16):
        headsel[h, h * 64 : (h + 1) * 64] = 1.0
    return headsel


def _core_masks(qi):
    """diagmask (64, NST*ST) {0,1} and biasmask (128, 16) for quarter qi."""
    diag = np.zeros((64, NST, ST), np.float32)
    if qi < 2:
        for st in range(NST):
            stg = 4 * qi + st
            pg = 64 * stg + np.arange(64)[:, None]
            sg = qi * SQ + st * ST + np.arange(ST)[None, :]
            diag[:, st, :] = np.where(sg >= 8 * pg + 8, 1.0, 0.0)
    else:
        diag[:] = 1.0
    diag = diag.reshape(64, NST * ST)
    diag = np.ascontiguousarray(np.concatenate([diag, diag], axis=0))

    bias = np.zeros((128, 16), np.float32)
    for st in range(NST):
        for pc in range(4):
            for pl in range(128):
                pp = 128 * pc + pl
                if qi == 1:
                    porig = pp + 256 if pp < 256 else pp - 256
                else:
                    porig = pp
                in_strip = qi < 2 and (64 * st <= pp < 64 * st + 64)
                if in_strip:
                    val = 0.0
                else:
                    s_min = qi * SQ + st * ST
                    val = 0.0 if s_min >= 8 * porig + 8 else NEG
                bias[pl, st * 4 + pc] = val
    return diag, bias


def _numpy_reference(x, lws, Wq, bq, Wk, bk, Wv, bv, Wo, bo):
    Bx, Sx, Dx = x.shape
    H, Hd, R = NUM_HEADS, HEAD_DIM, RATIO
    if lws <= R:
        return np.zeros_like(x)
    npool = lws // R
    trunc = npool * R
    comp = x[:, :trunc, :].reshape(Bx, npool, R, Dx).mean(axis=2)
    q = (x @ Wq + bq).reshape(Bx, Sx, H, Hd).transpose(0, 2, 1, 3)
    k = (comp @ Wk + bk).reshape(Bx, npool, H, Hd).transpose(0, 2, 1, 3)
    v = (comp @ Wv + bv).reshape(Bx, npool, H, Hd).transpose(0, 2, 1, 3)
    scores = np.einsum("bhqd,bhkd->bhqk", q, k) / np.sqrt(Hd)
    mask = np.arange(Sx)[:, None] >= (np.arange(npool) + 1) * R
    scores = np.where(mask[None, None], scores, -1e9)
    scores = scores - scores.max(axis=-1, keepdims=True)
    e = np.exp(scores)
    attn = e / e.sum(axis=-1, keepdims=True)
    out = np.einsum("bhqk,bhkd->bhqd", attn, v)
    out = out.transpose(0, 2, 1, 3).reshape(Bx, Sx, H * Hd)
    return (out @ Wo + bo).astype(np.float32)


def make_in_maps(x, Wq, bq, Wk, bk, Wv, bv, Wo, bo):
    from ml_dtypes import bfloat16

    x = np.asarray(x, np.float32)
    headsel = _host_constants()
    wqb = np.ascontiguousarray(Wq, np.float32).astype(bfloat16)
    # pooled k/v are computed from the *sum* over each window of 8; fold
    # the 1/8 mean scaling into the k/v projection weights here
    wkb = (np.ascontiguousarray(Wk, np.float32) / RATIO).astype(bfloat16)
    wvb = (np.ascontiguousarray(Wv, np.float32) / RATIO).astype(bfloat16)
    wob = np.ascontiguousarray(Wo, np.float32).astype(bfloat16)
    hsb = headsel.astype(bfloat16)
    bvr = np.asarray(bv, np.float32).reshape(1, D).astype(bfloat16)
    bor = np.asarray(bo, np.float32).reshape(1, D).astype(bfloat16)
    bq2 = np.ascontiguousarray(np.asarray(bq, np.float32).reshape(8, 128).T)
    bk2 = np.ascontiguousarray(np.asarray(bk, np.float32).reshape(8, 128).T)

    xb = x.astype(bfloat16)  # one bulk fp32->bf16 cast
    in_maps = []
    for core in range(N_CORES):
        b, qi = core // 4, core % 4
        xqT = np.ascontiguousarray(xb[b, qi * SQ : (qi + 1) * SQ, :].T)
        if qi == 1:
            xpc = np.concatenate([xb[b, 2048:4096], xb[b, 0:2048]], axis=0)
        else:
            xpc = xb[b, :LWS, :]
        xpT = np.ascontiguousarray(xpc.T)
        diag, bias = _core_masks(qi)
        in_maps.append(
            {
                "xqT": xqT,
                "xpT": xpT,
                "wq": wqb,
                "wk": wkb,
                "wv": wvb,
                "wo": wob,
                "bq2": bq2,
                "bk2": bk2,
                "bvr": bvr,
                "bor": bor,
                "headsel": hsb,
                "diagmask": diag.astype(bfloat16),
                "biasmask": bias,
            }
        )
    return in_maps


def assemble_output(x, Wv, bv, Wo, bo, results):
    y = np.empty((B, S, D), np.float32)
    for core in range(N_CORES):
        b, qi = core // 4, core % 4
        y[b, qi * SQ : (qi + 1) * SQ, :] = results[core]["y"]
    # rows 0..7: all pools masked -> reference uses uniform attention
    for b in range(B):
        vmean = x[b, :LWS, :].astype(np.float64).mean(axis=0).astype(np.float32)
        row = (vmean @ Wv + bv) @ Wo + bo
        y[b, 0:8, :] = row[None, :]
    return y


def kernel(**inputs):
    x = np.asarray(inputs["x"], np.float32)
    lws = int(np.asarray(inputs["local_window_start"]))
    Wq = np.asarray(inputs["Wq"], np.float32)
    bq = np.asarray(inputs["bq"], np.float32)
    Wk = np.asarray(inputs["Wk"], np.float32)
    bk = np.asarray(inputs["bk"], np.float32)
    Wv = np.asarray(inputs["Wv"], np.float32)
    bv = np.asarray(inputs["bv"], np.float32)
    Wo = np.asarray(inputs["Wo"], np.float32)
    bo = np.asarray(inputs["bo"], np.float32)

    if lws != LWS or x.shape != (B, S, D):
        return _numpy_reference(x, lws, Wq, bq, Wk, bk, Wv, bv, Wo, bo)

    try:
        _ensure_path()
        from concourse.bass_utils import run_bass_kernel_spmd

        global _RUNNER
        if _RUNNER is None:
            _RUNNER = build_program()
        nc = _RUNNER

        in_maps = make_in_maps(x, Wq, bq, Wk, bk, Wv, bv, Wo, bo)
        res = run_bass_kernel_spmd(nc, in_maps, list(range(N_CORES)))
        return assemble_output(x, Wv, bv, Wo, bo, res.results)
    except Exception as ex:  # device path unavailable -> correct host fallback
        sys.stderr.write(f"kernel: device path failed ({type(ex).__name__}: {ex}); "
                         "using host fallback\n")
        return _numpy_reference(x, lws, Wq, bq, Wk, bk, Wv, bv, Wo, bo)


if __name__ == "__main__":
    np.random.seed(0)
    xs = np.random.randn(B, S, D).astype(np.float32)
    sc = 1.0 / np.sqrt(D)
    args = dict(
        x=xs,
        local_window_start=LWS,
        Wq=np.random.randn(D, D).astype(np.float32) * sc,
        bq=np.zeros(D, np.float32),
        Wk=np.random.randn(D, D).astype(np.float32) * sc,
        bk=np.zeros(D, np.float32),
        Wv=np.random.randn(D, D).astype(np.float32) * sc,
        bv=np.zeros(D, np.float32),
        Wo=np.random.randn(D, D).astype(np.float32) * sc,
        bo=np.zeros(D, np.float32),
    )
    y = kernel(**args)
    ref = _numpy_reference(
        xs, LWS, args["Wq"], args["bq"], args["Wk"], args["bk"],
        args["Wv"], args["bv"], args["Wo"], args["bo"],
    )
    err = np.abs(y - ref)
    rel = err.max() / np.abs(ref).max()
    print("max abs err:", err.max(), "rel:", rel)
